# revision 28
# baseline (speedup 1.0000x reference)
"""Trainium2 Bass kernel for nn_DecoderBackbone (conv stem + 4 Mamba layers).

Sharding: 8 cores = 4 samples (batch) x 2-way tensor-parallel split of
d_inner (512 -> 256 per core).  Pair collectives over [[0,1],[2,3],[4,5],[6,7]].

Selective scan runs on the DVE tensor_tensor_scan primitive with lanes =
(d, n) on partitions (32 tiles of 128 lanes per core), time on the free axis.
"""

import sys

sys.path.insert(0, "/opt/trn_rl_repo")

import numpy as np

import concourse.bacc as bacc
import concourse.mybir as mybir
import concourse.tile as tile
from concourse.bass_utils import run_bass_kernel_spmd

F32 = mybir.dt.float32
BF16 = mybir.dt.bfloat16
ALU = mybir.AluOpType
ACTF = mybir.ActivationFunctionType
AX = mybir.AxisListType

# model dims
NL, HID, DIN, DST, DCONV, DTR = 4, 256, 512, 16, 4, 16
B, H, W = 4, 256, 256
L = 64 * 64          # tokens after conv stem
DH = DIN // 2        # d_inner half per core
NG = DH * DST // 128  # 32 lane tiles of 128 = (8 d) x (16 n)
NPIX1 = 128 * 128    # conv1 output pixels
N5 = 512             # matmul free chunk
TCH = 1024           # scan time chunk
NTCH = L // TCH
EPS = 1e-5

N_CORES = 8
GROUPS = [[0, 1], [2, 3], [4, 5], [6, 7]]

_cache = {}


def _mm(nc, out, lhsT, rhs, start=True, stop=True):
    nc.tensor.matmul(out, lhsT, rhs, start=start, stop=stop)


def _patch_act_tables():
    """Constrain the act-table chooser: Exp/Ln resolve only to the combined
    natural_log_exp table, Silu only to silu_and_others, so the scheduler's
    Exp/Ln interleaving stops thrashing table loads. Table ids stay
    positional; the real tables do contain the claimed functions."""
    import concourse.bacc as _bacc
    import concourse.hw_specs as _hw
    if getattr(_bacc, "_act_tables_patched", False):
        return
    orig = _hw.get_activation_tables

    def patched(module_arch):
        tabs = orig(module_arch)
        keep_exp_ln = "natural_log_exp_and_others"
        out = {}
        for name, funcs in tabs.items():
            f = set(funcs)
            if name != keep_exp_ln:
                f.discard(ACTF.Exp)
                f.discard(ACTF.Ln)
            out[name] = f
        return out

    _bacc.get_activation_tables = patched
    _bacc._act_tables_patched = True


def build_program():
    _patch_act_tables()
    nc = bacc.Bacc("TRN2", target_bir_lowering=False, debug=False,
                   num_devices=N_CORES)

    def din(name, shape, dtype):
        return nc.dram_tensor(name, shape, dtype, kind="ExternalInput")

    # ---- per-core external inputs (host-prepped layouts) ----
    x_cols_d = din("x_cols", [28, NPIX1], BF16)
    c1_lhsT_d = din("c1_lhsT", [28, 128], BF16)
    gn1_wb_d = din("gn1_wb", [128, 2], F32)
    c2_lhsT_d = din("c2_lhsT", [128, 9 * 128], BF16)
    c2_b_d = din("c2_b", [1, 128], BF16)
    gn2_wb_d = din("gn2_wb", [128, 2], F32)
    gb16_d = din("gb16", [128, 128], F32)
    gb32_d = din("gb32", [128, 128], F32)
    ones128_d = din("ones128", [128, 128], F32)
    yred_d = din("yred_lhsT", [128, 16 * 128], BF16)
    ipw_d = [din(f"ipw_{l}", [128, 2 * 512], BF16) for l in range(NL)]
    xzb_d = [din(f"xzb_{l}", [128, 4], F32) for l in range(NL)]
    cwdg_d = [din(f"cwdg_{l}", [128, 8 * 128], BF16) for l in range(NL)]
    cb_d = [din(f"cb_{l}", [128, 2], F32) for l in range(NL)]
    xpw_d = [din(f"xpw_{l}", [128, 4 * 272], BF16) for l in range(NL)]
    dpw_d = [din(f"dpw_{l}", [16, 256], BF16) for l in range(NL)]
    dpb_d = [din(f"dpb_{l}", [128, 2], F32) for l in range(NL)]
    dalh_d = [din(f"dalh_{l}", [128, 32 * 128], BF16) for l in range(NL)]
    acol_d = [din(f"acol_{l}", [128, 32], F32) for l in range(NL)]
    dd_d = [din(f"dd_{l}", [128, 2], F32) for l in range(NL)]
    opw_d = [din(f"opw_{l}", [128, 2 * 256], BF16) for l in range(NL)]

    s_out_d = nc.dram_tensor("s_out", [HID, L], F32, kind="ExternalOutput")

    with tile.TileContext(nc) as tc:
        with tc.tile_pool(name="persist", bufs=1) as pp:
            gb16 = pp.tile([128, 128], F32, tag="gb16")
            nc.sync.dma_start(gb16[:], gb16_d.ap())
            gb32 = pp.tile([128, 128], F32, tag="gb32")
            nc.sync.dma_start(gb32[:], gb32_d.ap())
            ones128 = pp.tile([128, 128], F32, tag="ones128")
            nc.sync.dma_start(ones128[:], ones128_d.ap())
            yred = pp.tile([128, 16 * 128], BF16, tag="yred")
            nc.sync.dma_start(yred[:], yred_d.ap())
            ones_row = pp.tile([1, N5], BF16, tag="ones_row")
            nc.vector.memset(ones_row[:], 1.0)
            eps_col = pp.tile([128, 1], F32, tag="eps_col")
            nc.vector.memset(eps_col[:], EPS)

            # residual stream sT [HID, L] fp32 as 2 partition tiles
            s_t = [pp.tile([128, L], F32, tag=f"s{k}", name=f"s_{k}") for k in range(2)]

            with tc.tile_pool(name="dram", bufs=1, space="DRAM") as dp:
                s_cc_in = dp.tile([128, L], F32, tag="s_cc_in")
                s_cc_out = dp.tile([HID, L], F32, tag="s_cc_out")
                u_cc_in = dp.tile([DH, L], BF16, tag="u_cc_in")
                u_cc_out = dp.tile([DIN, L], BF16, tag="u_cc_out")
                o_cc_in = dp.tile([NTCH * HID, TCH], F32, tag="o_cc_in")
                o_cc_out = dp.tile([NTCH * HID, TCH], F32, tag="o_cc_out")

                build_conv_stem(nc, tc, x_cols_d, c1_lhsT_d, gn1_wb_d,
                                c2_lhsT_d, c2_b_d, gn2_wb_d, gb16, gb32,
                                ones_row, eps_col, s_t, s_cc_in, s_cc_out)

                for l in range(NL):
                    build_layer(nc, tc, l, s_t, ones128, yred, eps_col,
                                ipw_d[l], xzb_d[l], cwdg_d[l], cb_d[l],
                                xpw_d[l],
                                dpw_d[l], dpb_d[l], dalh_d[l], acol_d[l],
                                dd_d[l],
                                opw_d[l], u_cc_in, u_cc_out, o_cc_in,
                                o_cc_out)

            for k in range(2):
                nc.sync.dma_start(s_out_d.ap()[k * 128:(k + 1) * 128, :],
                                  s_t[k][:])

    nc.compile()
    return nc


def group_norm_scale_bias(nc, pool, gsum_ps, wb, inv_n, eps_col):
    """From group-replicated [128,2] psum (sum, sumsq) -> per-partition
    scale/bias columns for the fused silu(gn) activation."""
    m = pool.tile([128, 1], F32, tag="gn_m")
    nc.vector.tensor_scalar(m[:], gsum_ps[:, 0:1], inv_n, None, ALU.mult)
    m2 = pool.tile([128, 1], F32, tag="gn_m2")
    nc.scalar.square(m2[:], m[:])
    var = pool.tile([128, 1], F32, tag="gn_var")
    nc.vector.scalar_tensor_tensor(var[:], gsum_ps[:, 1:2], inv_n, m2[:],
                                   ALU.mult, ALU.subtract)
    lnv = pool.tile([128, 1], F32, tag="gn_lnv")
    nc.scalar.activation(lnv[:], var[:], ACTF.Ln, bias=eps_col[:])
    inv = pool.tile([128, 1], F32, tag="gn_inv")
    nc.scalar.activation(inv[:], lnv[:], ACTF.Exp, scale=-0.5)
    scale = pool.tile([128, 1], F32, tag="gn_scale")
    nc.vector.tensor_tensor(scale[:], inv[:], wb[:, 0:1], ALU.mult)
    nscale = pool.tile([128, 1], F32, tag="gn_nscale")
    nc.vector.tensor_scalar(nscale[:], scale[:], -1.0, None, ALU.mult)
    bias = pool.tile([128, 1], F32, tag="gn_bias")
    nc.vector.scalar_tensor_tensor(bias[:], m[:], nscale[:], wb[:, 1:2],
                                   ALU.mult, ALU.add)
    return scale, bias


def build_conv_stem(nc, tc, x_cols_d, c1_lhsT_d, gn1_wb_d, c2_lhsT_d, c2_b_d,
                    gn2_wb_d, gb16, gb32, ones_row, eps_col, s_t, s_cc_in,
                    s_cc_out):
    from contextlib import ExitStack
    es = ExitStack()
    cp = es.enter_context(tc.tile_pool(name="conv_w", bufs=1))
    psp = es.enter_context(tc.tile_pool(name="conv_ps", bufs=2, space="PSUM"))
    scp = es.enter_context(tc.tile_pool(name="conv_sc", bufs=2))
    c1_lhsT = cp.tile([28, 128], BF16, tag="c1w")
    nc.sync.dma_start(c1_lhsT[:], c1_lhsT_d.ap())
    gn1_wb = cp.tile([128, 2], F32, tag="gn1wb")
    nc.sync.dma_start(gn1_wb[:], gn1_wb_d.ap())

    sums = cp.tile([128, 32], F32, tag="c1_sums")
    sqs = cp.tile([128, 32], F32, tag="c1_sqs")

    es_y1 = ExitStack()
    padp = es_y1.enter_context(tc.tile_pool(name="conv_pad0", bufs=1))
    # padded layout [128, 129*129]: row 0 / col 0 are zero padding
    y1pad = padp.tile([128, 129 * 129], BF16, tag="y1pad")
    y1p = y1pad[:].rearrange("p (h w) -> p h w", h=129)
    nc.vector.memset(y1p[:, 0:1, :], 0.0)
    nc.vector.memset(y1p[:, :, 0:1], 0.0)
    with tc.tile_pool(name="conv_x", bufs=1) as xp_pool:
        x_cols = xp_pool.tile([28, NPIX1], BF16, tag="x_cols")
        nc.sync.dma_start(x_cols[:], x_cols_d.ap())
        for n in range(NPIX1 // N5):
            ps = psp.tile([128, N5], F32, tag="c_ps")
            _mm(nc, ps[:], c1_lhsT[:], x_cols[:, n * N5:(n + 1) * N5])
            r0, c0 = divmod(n * N5, 128)
            dst = y1p[:, 1 + r0:1 + r0 + 4, 1:129]
            nc.scalar.activation(dst, ps[:], ACTF.Copy,
                                 accum_out=sums[:, n:n + 1])
            scr = scp.tile([128, N5], F32, tag="c_scr")
            nc.scalar.activation(scr[:], ps[:], ACTF.Square,
                                 accum_out=sqs[:, n:n + 1])

    stats = cp.tile([128, 2], F32, tag="c_stats")
    nc.vector.tensor_reduce(stats[:, 0:1], sums[:], AX.X, ALU.add)
    nc.vector.tensor_reduce(stats[:, 1:2], sqs[:], AX.X, ALU.add)
    gs_ps = psp.tile([128, 2], F32, tag="c_gs")
    _mm(nc, gs_ps[:], gb16[:], stats[:])
    scale, bias = group_norm_scale_bias(nc, cp, gs_ps, gn1_wb,
                                        1.0 / (16 * NPIX1), eps_col)

    if True:
        gn1_view = y1p[:, 1:129, 1:129]
        nc.scalar.activation(gn1_view, gn1_view, ACTF.Silu,
                             bias=bias[:], scale=scale[:])

        # ---- conv2 (oc half per core), stride 2, 3x3 ----
        c2_lhsT = cp.tile([128, 9 * 128], BF16, tag="c2w")
        nc.sync.dma_start(c2_lhsT[:], c2_lhsT_d.ap())
        c2_b = cp.tile([1, 128], BF16, tag="c2b")
        nc.sync.dma_start(c2_b[:], c2_b_d.ap())
        gn2_wb = cp.tile([128, 2], F32, tag="gn2wb")
        nc.sync.dma_start(gn2_wb[:], gn2_wb_d.ap())

        y2_raw = cp.tile([128, L], F32, tag="y2_raw")
        sums2 = cp.tile([128, 8], F32, tag="c2_sums")
        sqs2 = cp.tile([128, 8], F32, tag="c2_sqs")
        for n in range(L // N5):
            oh0 = n * 8
            ps = psp.tile([128, N5], F32, tag="c_ps")
            first = True
            for kh in range(3):
                for kw in range(3):
                    rhs = y1p[:, kh + 2 * oh0: kh + 2 * oh0 + 15: 2,
                              kw: kw + 127: 2]
                    _mm(nc, ps[:], c2_lhsT[:, (kh * 3 + kw) * 128:
                                           (kh * 3 + kw + 1) * 128],
                        rhs, start=first, stop=False)
                    first = False
            _mm(nc, ps[:], c2_b[:], ones_row[:], start=False, stop=True)
            nc.scalar.activation(y2_raw[:, n * N5:(n + 1) * N5], ps[:],
                                 ACTF.Copy, accum_out=sums2[:, n:n + 1])
            scr = scp.tile([128, N5], F32, tag="c_scr")
            nc.scalar.activation(scr[:], ps[:], ACTF.Square,
                                 accum_out=sqs2[:, n:n + 1])
        es_y1.close()  # y1pad dead

    stats2 = cp.tile([128, 2], F32, tag="c_stats2")
    nc.vector.tensor_reduce(stats2[:, 0:1], sums2[:], AX.X, ALU.add)
    nc.vector.tensor_reduce(stats2[:, 1:2], sqs2[:], AX.X, ALU.add)
    gs2_ps = psp.tile([128, 2], F32, tag="c_gs")
    _mm(nc, gs2_ps[:], gb32[:], stats2[:])
    scale2, bias2 = group_norm_scale_bias(nc, cp, gs2_ps, gn2_wb,
                                          1.0 / (32 * L), eps_col)
    nc.scalar.activation(y2_raw[:], y2_raw[:], ACTF.Silu, bias=bias2[:],
                         scale=scale2[:])
    s_own = y2_raw

    # exchange halves -> full sT on both cores of the pair
    nc.sync.dma_start(s_cc_in[:], s_own[:])
    nc.gpsimd.collective_compute(
        "AllGather", ALU.bypass, replica_groups=GROUPS,
        ins=[s_cc_in.opt()], outs=[s_cc_out.opt()])
    for k in range(2):
        nc.sync.dma_start(s_t[k][:], s_cc_out[k * 128:(k + 1) * 128, :])
    es.close()


def build_layer(nc, tc, l, s_t, ones128, yred, eps_col, ipw_d, xzb_d, cwdg_d,
                cb_d, xpw_d,
                dpw_d, dpb_d, dalh_d, acol_d, dd_d, opw_d, u_cc_in,
                u_cc_out, o_cc_in, o_cc_out):
    with tc.tile_pool(name=f"lw{l}", bufs=1) as wp:
        xzb = wp.tile([128, 4], F32, tag="xzb")
        nc.sync.dma_start(xzb[:], xzb_d.ap())
        cwdg = wp.tile([128, 8 * 128], BF16, tag="cwdg")
        nc.sync.dma_start(cwdg[:], cwdg_d.ap())
        cb = wp.tile([128, 2], F32, tag="cb")
        nc.sync.dma_start(cb[:], cb_d.ap())
        xpw = wp.tile([128, 4 * 272], BF16, tag="xpw")
        nc.sync.dma_start(xpw[:], xpw_d.ap())
        dpw = wp.tile([16, 256], BF16, tag="dpw")
        nc.sync.dma_start(dpw[:], dpw_d.ap())
        dpb = wp.tile([128, 2], F32, tag="dpb")
        nc.sync.dma_start(dpb[:], dpb_d.ap())
        dalh = wp.tile([128, 32 * 128], BF16, tag="dalh")
        nc.sync.dma_start(dalh[:], dalh_d.ap())
        dd = wp.tile([128, 2], F32, tag="dd")
        nc.sync.dma_start(dd[:], dd_d.ap())
        opw = wp.tile([128, 2 * 256], BF16, tag="opw")
        nc.sync.dma_start(opw[:], opw_d.ap())

        with tc.tile_pool(name=f"lact{l}", bufs=1) as ap:
            z_t = [ap.tile([128, L], BF16, tag=f"z{k}", name=f"z_{l}_{k}") for k in range(2)]
            u_own = [ap.tile([128, L], BF16, tag=f"uo{k}", name=f"uo_{l}_{k}") for k in range(2)]
            hstate = ap.tile([128, NG], F32, tag="hstate")
            bt = ap.tile([128, L], BF16, tag="bt")
            ct = ap.tile([128, L], BF16, tag="ct")
            dtt = ap.tile([16, L], BF16, tag="dtt")

            # ---- phase A: LN + in_proj + conv1d + u-gather ----
            phase_a(nc, tc, l, s_t, ones128, eps_col, ipw_d, xzb, cwdg, cb,
                    u_own, z_t, u_cc_in, u_cc_out)

            # ---- phase B: x_proj -> dt | B~ | C~ (whole L) ----
            with tc.tile_pool(name=f"B{l}", bufs=2) as pb, \
                 tc.tile_pool(name=f"Bps{l}", bufs=2, space="PSUM") as pbp:
                for n in range(L // N5):
                    c = slice(n * N5, (n + 1) * N5)
                    us = []
                    for k in range(4):
                        ut = pb.tile([128, N5], BF16, tag=f"us{k}")
                        nc.sync.dma_start(
                            ut[:], u_cc_out[k * 128:(k + 1) * 128, c])
                        us.append(ut)
                    for mi, (m0, m1, dst) in enumerate(
                            [(0, 16, dtt), (16, 144, bt), (144, 272, ct)]):
                        ps = pbp.tile([m1 - m0, N5], F32, tag=f"ps{mi}")
                        for k in range(4):
                            _mm(nc, ps[:],
                                xpw[:, k * 272 + m0:k * 272 + m1], us[k][:],
                                start=(k == 0), stop=(k == 3))
                        nc.scalar.activation(dst[:, c], ps[:], ACTF.Copy)

            # ---- phase C: delta + scan stack ----
            phase_c(nc, tc, l, dtt, dpw, dpb, bt, ct, hstate, dalh, yred,
                    dd, opw, u_own, z_t, o_cc_in, o_cc_out, s_t)



def phase_a(nc, tc, l, s_t, ones128, eps_col, ipw_d, xzb, cwdg, cb, u_own,
            z_t, u_cc_in, u_cc_out):
    with tc.tile_pool(name=f"A{l}", bufs=2) as pa, \
         tc.tile_pool(name=f"Ab{l}", bufs=1) as pa1, \
         tc.tile_pool(name=f"Aps{l}", bufs=2, space="PSUM") as pap:
        ipw = pa1.tile([128, 2 * 512], BF16, tag="ipw")
        nc.sync.dma_start(ipw[:], ipw_d.ap())
        xc_lin = [pa1.tile([128, 3 + L], BF16, tag=f"xcl{k}",
                               name=f"xcl_{l}_{k}") for k in range(2)]
        for k in range(2):
            nc.vector.memset(xc_lin[k][:, 0:3], 0.0)
        for n in range(L // N5):
            c = slice(n * N5, (n + 1) * N5)
            ps_m = pap.tile([128, N5], F32, tag="ln_m")
            _mm(nc, ps_m[:], ones128[:], s_t[0][:, c], start=True,
                stop=False)
            _mm(nc, ps_m[:], ones128[:], s_t[1][:, c], start=False,
                stop=True)
            ps_q = pap.tile([128, N5], F32, tag="ln_q")
            for k in range(2):
                sq = pa.tile([128, N5], F32, tag="ln_sq")
                nc.vector.tensor_tensor(sq[:], s_t[k][:, c], s_t[k][:, c],
                                        ALU.mult)
                _mm(nc, ps_q[:], ones128[:], sq[:], start=(k == 0),
                    stop=(k == 1))
            m2s = pa.tile([128, N5], F32, tag="ln_m2s")
            nc.scalar.activation(m2s[:], ps_m[:], ACTF.Square,
                                 scale=1.0 / HID)
            var = pa.tile([128, N5], F32, tag="ln_var")
            nc.vector.scalar_tensor_tensor(var[:], ps_q[:], 1.0 / HID,
                                           m2s[:], ALU.mult, ALU.subtract)
            lnv = pa.tile([128, N5], F32, tag="ln_lnv")
            nc.scalar.activation(lnv[:], var[:], ACTF.Ln, bias=eps_col[:])
            inv = pa.tile([128, N5], F32, tag="ln_inv")
            nc.scalar.activation(inv[:], lnv[:], ACTF.Exp, scale=-0.5)
            xn = []
            for k in range(2):
                t1 = pa.tile([128, N5], F32, tag="ln_t1")
                nc.vector.scalar_tensor_tensor(
                    t1[:], ps_m[:], -1.0 / HID, s_t[k][:, c], ALU.mult,
                    ALU.add)
                xnk = pa.tile([128, N5], BF16, tag="ln_xn")
                nc.vector.tensor_tensor(xnk[:], t1[:], inv[:], ALU.mult)
                xn.append(xnk)
            # in_proj: m-tiles 0,1 = xc halves; 2,3 = z halves
            for m in range(4):
                ps = pap.tile([128, N5], F32, tag="xz_ps")
                for k in range(2):
                    _mm(nc, ps[:],
                        ipw[:, k * 512 + m * 128:k * 512 + (m + 1) * 128],
                        xn[k][:], start=(k == 0), stop=(k == 1))
                if m < 2:
                    nc.scalar.activation(
                        xc_lin[m][:, 3 + n * N5: 3 + (n + 1) * N5], ps[:],
                        ACTF.Identity, bias=xzb[:, m:m + 1])
                else:
                    nc.scalar.activation(z_t[m - 2][:, c], ps[:],
                                         ACTF.Identity,
                                         bias=xzb[:, m:m + 1])
        # conv1d (depthwise, 4 causal taps) as PE diagonal matmuls,
        # silu fused into the psum evacuation
        for k in range(2):
            for n in range(L // N5):
                ps_u = pap.tile([128, N5], F32, tag="cv_ps")
                for j in range(4):
                    _mm(nc, ps_u[:],
                        cwdg[:, (k * 4 + j) * 128:(k * 4 + j + 1) * 128],
                        xc_lin[k][:, j + n * N5: j + (n + 1) * N5],
                        start=(j == 0), stop=(j == 3))
                nc.scalar.activation(u_own[k][:, n * N5:(n + 1) * N5],
                                     ps_u[:], ACTF.Identity,
                                     bias=cb[:, k:k + 1])
        for k in range(2):
            nc.scalar.activation(u_own[k][:], u_own[k][:], ACTF.Silu)
            nc.scalar.activation(z_t[k][:], z_t[k][:], ACTF.Silu)
            nc.sync.dma_start(u_cc_in[k * 128:(k + 1) * 128, :],
                              u_own[k][:])
    nc.gpsimd.collective_compute(
        "AllGather", ALU.bypass, replica_groups=GROUPS,
        ins=[u_cc_in.opt()], outs=[u_cc_out.opt()])


def phase_c(nc, tc, l, dtt, dpw, dpb, bt, ct, hstate, dalh, yred, dd, opw,
            u_own, z_t, o_cc_in, o_cc_out, s_t):
    with tc.tile_pool(name=f"C{l}", bufs=3) as pc, \
         tc.tile_pool(name=f"Ch{l}", bufs=9) as phc, \
         tc.tile_pool(name=f"Cu{l}", bufs=6) as pdur, \
         tc.tile_pool(name=f"Cd{l}", bufs=2) as pdl, \
         tc.tile_pool(name=f"Cr{l}", bufs=2) as pr, \
         tc.tile_pool(name=f"Cps{l}", bufs=3, space="PSUM") as pcp, \
         tc.tile_pool(name=f"Cp1{l}", bufs=1, space="PSUM") as pcp1, \
         tc.tile_pool(name=f"Cyps{l}", bufs=1, space="PSUM") as pyp:

        def delta_du(t):
            ts_l = slice(t * TCH, (t + 1) * TCH)
            dl, duc = [], []
            for k in range(2):
                dlk = pdl.tile([128, TCH], BF16, tag=f"dl{k}",
                               name=f"dl_{l}_{t}_{k}")
                for j in range(TCH // N5):
                    cj = slice(t * TCH + j * N5, t * TCH + (j + 1) * N5)
                    ps_d = pcp1.tile([128, N5], F32, tag="ps_so")
                    _mm(nc, ps_d[:], dpw[:, k * 128:(k + 1) * 128],
                        dtt[:, cj])
                    es_t = pdl.tile([128, N5], F32, tag="sp_e")
                    nc.scalar.activation(es_t[:], ps_d[:], ACTF.Exp,
                                         bias=dpb[:, k:k + 1])
                    nc.scalar.activation(dlk[:, j * N5:(j + 1) * N5],
                                         es_t[:], ACTF.Ln, bias=1.0)
                duk = pdl.tile([128, TCH], BF16, tag=f"du{k}",
                               name=f"du_{l}_{t}_{k}")
                nc.vector.tensor_tensor(duk[:], dlk[:], u_own[k][:, ts_l],
                                        ALU.mult)
                dl.append(dlk)
                duc.append(duk)
            return dl, duc

        nxt = delta_du(0)
        deferred = []
        for t in range(NTCH):
            ts = slice(t * TCH, (t + 1) * TCH)
            dl, duc = nxt
            if t + 1 < NTCH:
                with tc.high_priority(offset=400):
                    nxt = delta_du(t + 1)
            ps_y = [pyp.tile([128, TCH], F32, tag=f"ps_y{pt}",
                             name=f"ps_y_{l}_{t}_{pt}") for pt in range(2)]
            hc_batch = []
            for g in range(NG):
                kt = g // 16
                r0 = (8 * g) % 128
                # dA = exp(A * delta) via one-hot*A matmul + exp
                da = pc.tile([128, TCH], BF16, tag="da")
                for j in range(TCH // N5):
                    ps_da = pcp.tile([128, N5], F32, tag="ps_da")
                    _mm(nc, ps_da[:],
                        dalh[:, g * 128:(g + 1) * 128],
                        dl[kt][:, j * N5:(j + 1) * N5])
                    nc.scalar.activation(da[:, j * N5:(j + 1) * N5],
                                         ps_da[:], ACTF.Exp)
                # du_rep via broadcast DMA, dBu = du_rep * B~
                dur = pdur.tile([128, TCH], BF16, tag="dur")
                src = duc[kt][r0:r0 + 8, :].unsqueeze(1)
                nc.gpsimd.dma_start(dur[:], src.broadcast_to([8, 16, TCH]))
                dbu = pc.tile([128, TCH], BF16, tag="dbu")
                nc.vector.tensor_tensor(dbu[:], dur[:], bt[:, ts], ALU.mult)
                # selective scan
                h = pc.tile([128, TCH], BF16, tag="h")
                init = 0.0 if t == 0 else hstate[:, g:g + 1]
                nc.vector.tensor_tensor_scan(h[:], da[:], dbu[:], init,
                                             ALU.mult, ALU.add)
                nc.vector.tensor_copy(hstate[:, g:g + 1],
                                      h[:, TCH - 1:TCH])
                # y partial: sum_n h*C; yred matmuls emitted in batches of
                # 8 lane-tiles so the PE stream is never head-of-line
                # blocked waiting for a single hc
                hc = phc.tile([128, TCH], BF16, tag="hc")
                nc.vector.tensor_tensor(hc[:], h[:], ct[:, ts], ALU.mult)
                hc_batch.append((g, kt, hc))
                if len(hc_batch) == 8:
                    for gg, kkt, hhc in hc_batch:
                        for j in range(TCH // N5):
                            _mm(nc, ps_y[kkt][:, j * N5:(j + 1) * N5],
                                yred[:, (gg % 16) * 128:
                                     (gg % 16 + 1) * 128],
                                hhc[:, j * N5:(j + 1) * N5],
                                start=(gg % 16 == 0), stop=(gg % 16 == 15))
                    hc_batch = []
            # y = ys + u*D ; r = y * silu(z) ; out_proj partial
            rt = []
            for k in range(2):
                y = pr.tile([128, TCH], BF16, tag=f"y{k}")
                nc.vector.scalar_tensor_tensor(
                    y[:], u_own[k][:, ts], dd[:, k:k + 1], ps_y[k][:],
                    ALU.mult, ALU.add)
                r = pr.tile([128, TCH], BF16, tag=f"r{k}")
                nc.vector.tensor_tensor(r[:], y[:], z_t[k][:, ts], ALU.mult)
                rt.append(r)
            for m in range(2):
                for j in range(TCH // N5):
                    ps_o = pcp1.tile([128, N5], F32, tag="ps_so")
                    for k in range(2):
                        _mm(nc, ps_o[:],
                            opw[:, k * 256 + m * 128:
                                k * 256 + (m + 1) * 128],
                            rt[k][:, j * N5:(j + 1) * N5],
                            start=(k == 0), stop=(k == 1))
                    o_sb = pr.tile([128, N5], F32, tag="o_sb")
                    nc.scalar.activation(o_sb[:], ps_o[:], ACTF.Copy)
                    nc.sync.dma_start(
                        o_cc_in[t * HID + m * 128:t * HID + (m + 1) * 128,
                                j * N5:(j + 1) * N5],
                        o_sb[:])
            # launch the pair all-reduce now; apply residual at layer end
            nc.gpsimd.collective_compute(
                "AllReduce", ALU.add, replica_groups=GROUPS,
                ins=[o_cc_in[t * HID:(t + 1) * HID, :].opt()],
                outs=[o_cc_out[t * HID:(t + 1) * HID, :].opt()])
            deferred.append(t)
        for t in deferred:
            ts = slice(t * TCH, (t + 1) * TCH)
            for k in range(2):
                o = pr.tile([128, TCH], F32, tag="o_in")
                nc.gpsimd.dma_start(
                    o[:], o_cc_out[t * HID + k * 128:
                                   t * HID + (k + 1) * 128, :])
                nc.vector.tensor_tensor(s_t[k][:, ts], s_t[k][:, ts],
                                        o[:], ALU.add)


# ======================= host side =======================

def _prep_inputs(inputs):
    """Host-side weight layout prep. Returns per-core input maps."""
    f32 = np.float32
    import ml_dtypes
    bf16 = ml_dtypes.bfloat16

    def to_bf(a):
        return np.asarray(a, dtype=f32).astype(bf16)

    x = np.asarray(inputs["x"], f32)
    conv1_w = np.asarray(inputs["conv1_w"], f32)
    conv1_b = np.asarray(inputs["conv1_b"], f32)
    gn1_w = np.asarray(inputs["gn1_w"], f32)
    gn1_b = np.asarray(inputs["gn1_b"], f32)
    conv2_w = np.asarray(inputs["conv2_w"], f32)
    conv2_b = np.asarray(inputs["conv2_b"], f32)
    gn2_w = np.asarray(inputs["gn2_w"], f32)
    gn2_b = np.asarray(inputs["gn2_b"], f32)
    ln_w = np.asarray(inputs["ln_w"], f32)
    ln_b = np.asarray(inputs["ln_b"], f32)
    in_proj_w = np.asarray(inputs["in_proj_w"], f32)
    conv1d_w = np.asarray(inputs["conv1d_w"], f32)
    conv1d_b = np.asarray(inputs["conv1d_b"], f32)
    x_proj_w = np.asarray(inputs["x_proj_w"], f32)
    dt_proj_w = np.asarray(inputs["dt_proj_w"], f32)
    dt_proj_b = np.asarray(inputs["dt_proj_b"], f32)
    A_log = np.asarray(inputs["A_log"], f32)
    Dp = np.asarray(inputs["D"], f32)
    out_proj_w = np.asarray(inputs["out_proj_w"], f32)

    # --- conv1 im2col per sample: rows (c,kh,kw) + ones row ---
    xp = np.zeros((B, 3, H + 2, W + 2), f32)
    xp[:, :, 1:H + 1, 1:W + 1] = x
    cols = []
    for c in range(3):
        for kh in range(3):
            for kw in range(3):
                v = xp[:, c, kh:kh + 2 * 128:2, kw:kw + 2 * 128:2]
                cols.append(v.reshape(B, -1))
    x_cols = np.stack(cols, axis=1)  # [B, 27, 16384]
    x_cols = np.concatenate(
        [x_cols, np.ones((B, 1, NPIX1), f32)], axis=1)

    c1_lhsT = np.concatenate(
        [conv1_w.reshape(128, 27).T, conv1_b[None, :]], axis=0)  # [28,128]
    gn1_wb = np.stack([gn1_w, gn1_b], axis=1)

    def gblock(gsz):
        m = np.zeros((128, 128), f32)
        for i in range(128):
            g0 = (i // gsz) * gsz
            m[g0:g0 + gsz, i] = 1.0
        return m

    gb16, gb32 = gblock(16), gblock(32)
    ones128 = np.ones((128, 128), f32)

    yred = np.zeros((16, 128, 128), f32)
    for j in range(16):
        for p in range(128):
            yred[j, p, 8 * j + p // 16] = 1.0
    yred_l = to_bf(yred.transpose(1, 0, 2).reshape(128, 16 * 128))

    shared = dict(gb16=gb16, gb32=gb32, ones128=ones128, yred_lhsT=yred_l)

    in_maps = []
    for core in range(N_CORES):
        b, hh = core // 2, core % 2
        dsl = slice(hh * DH, (hh + 1) * DH)
        m = dict(shared)
        m["x_cols"] = to_bf(x_cols[b])
        m["c1_lhsT"] = to_bf(c1_lhsT)
        m["gn1_wb"] = gn1_wb
        ocs = slice(hh * 128, (hh + 1) * 128)
        c2 = np.zeros((128, 9 * 128), f32)
        for kh in range(3):
            for kw in range(3):
                tap = kh * 3 + kw
                c2[:, tap * 128:(tap + 1) * 128] = conv2_w[ocs, :, kh, kw].T
        m["c2_lhsT"] = to_bf(c2)
        m["c2_b"] = to_bf(conv2_b[None, ocs])
        m["gn2_wb"] = np.stack([gn2_w[ocs], gn2_b[ocs]], axis=1)

        for l in range(NL):
            ipw = in_proj_w[l] * ln_w[l][None, :]
            xzb = in_proj_w[l] @ ln_b[l]
            rows = np.concatenate(
                [np.arange(hh * DH, (hh + 1) * DH),
                 np.arange(DIN + hh * DH, DIN + (hh + 1) * DH)])
            ipw_h = ipw[rows]
            ipw_l = np.zeros((128, 2 * 512), f32)
            for k in range(2):
                ipw_l[:, k * 512:(k + 1) * 512] = \
                    ipw_h[:, k * 128:(k + 1) * 128].T
            m[f"ipw_{l}"] = to_bf(ipw_l)
            xzb_h = xzb[rows]
            m[f"xzb_{l}"] = np.stack(
                [xzb_h[0:128], xzb_h[128:256], xzb_h[256:384],
                 xzb_h[384:512]], axis=1)
            cw = conv1d_w[l, dsl, 0, :]
            cbv = conv1d_b[l, dsl]
            cwdg = np.zeros((128, 8 * 128), f32)
            for k in range(2):
                for j in range(4):
                    cwdg[:, (k * 4 + j) * 128:(k * 4 + j + 1) * 128] = \
                        np.diag(cw[k * 128:(k + 1) * 128, j])
            m[f"cwdg_{l}"] = to_bf(cwdg)
            m[f"cb_{l}"] = np.stack([cbv[0:128], cbv[128:256]], axis=1)
            xa = np.zeros((DIN, 272), f32)
            xa[:, 0:16] = x_proj_w[l, 0:DTR, :].T
            nidx = np.arange(128) % 16
            xa[:, 16:144] = x_proj_w[l, DTR + nidx, :].T
            xa[:, 144:272] = x_proj_w[l, DTR + DST + nidx, :].T
            xpw_l = np.zeros((128, 4 * 272), f32)
            for k in range(4):
                xpw_l[:, k * 272:(k + 1) * 272] = xa[k * 128:(k + 1) * 128]
            m[f"xpw_{l}"] = to_bf(xpw_l)
            m[f"dpw_{l}"] = to_bf(dt_proj_w[l, dsl, :].T)
            dpb_h = dt_proj_b[l, dsl]
            m[f"dpb_{l}"] = np.stack([dpb_h[0:128], dpb_h[128:256]], axis=1)
            A = -np.exp(A_log[l, dsl, :])
            dal = np.zeros((32, 128, 128), f32)
            for g in range(32):
                r0 = (8 * g) % 128
                for p in range(128):
                    dal[g, r0 + p // 16, p] = A[8 * g + p // 16, p % 16]
            m[f"dalh_{l}"] = to_bf(
                dal.transpose(1, 0, 2).reshape(128, 32 * 128))
            ac = np.zeros((128, 32), f32)
            for g in range(32):
                for p in range(128):
                    ac[p, g] = A[8 * g + p // 16, p % 16]
            m[f"acol_{l}"] = ac
            Dh = Dp[l, dsl]
            m[f"dd_{l}"] = np.stack([Dh[0:128], Dh[128:256]], axis=1)
            opw_h = out_proj_w[l][:, dsl]
            opw_l = np.zeros((128, 2 * 256), f32)
            for k in range(2):
                opw_l[:, k * 256:(k + 1) * 256] = \
                    opw_h[:, k * 128:(k + 1) * 128].T
            m[f"opw_{l}"] = to_bf(opw_l)
        in_maps.append({k: np.ascontiguousarray(v) for k, v in m.items()})
    return in_maps


def kernel(**inputs):
    if "nc" not in _cache:
        _cache["nc"] = build_program()
    nc = _cache["nc"]
    in_maps = _prep_inputs(inputs)
    res = run_bass_kernel_spmd(nc, in_maps, list(range(N_CORES)))
    s = np.stack([np.asarray(res.results[2 * b]["s_out"], np.float32).T
                  for b in range(B)])
    return (s, 64, 64)


# revision 29
# speedup vs baseline: 1.0025x; 1.0025x over previous
"""Trainium2 Bass kernel for nn_DecoderBackbone (conv stem + 4 Mamba layers).

Sharding: 8 cores = 4 samples (batch) x 2-way tensor-parallel split of
d_inner (512 -> 256 per core).  Pair collectives over [[0,1],[2,3],[4,5],[6,7]].

Selective scan runs on the DVE tensor_tensor_scan primitive with lanes =
(d, n) on partitions (32 tiles of 128 lanes per core), time on the free axis.
"""

import sys

sys.path.insert(0, "/opt/trn_rl_repo")

import numpy as np

import concourse.bacc as bacc
import concourse.mybir as mybir
import concourse.tile as tile
from concourse.bass_utils import run_bass_kernel_spmd

F32 = mybir.dt.float32
BF16 = mybir.dt.bfloat16
ALU = mybir.AluOpType
ACTF = mybir.ActivationFunctionType
AX = mybir.AxisListType

# model dims
NL, HID, DIN, DST, DCONV, DTR = 4, 256, 512, 16, 4, 16
B, H, W = 4, 256, 256
L = 64 * 64          # tokens after conv stem
DH = DIN // 2        # d_inner half per core
NG = DH * DST // 128  # 32 lane tiles of 128 = (8 d) x (16 n)
NPIX1 = 128 * 128    # conv1 output pixels
N5 = 512             # matmul free chunk
TCH = 1024           # scan time chunk
NTCH = L // TCH
EPS = 1e-5

N_CORES = 8
GROUPS = [[0, 1], [2, 3], [4, 5], [6, 7]]

_cache = {}


def _mm(nc, out, lhsT, rhs, start=True, stop=True):
    nc.tensor.matmul(out, lhsT, rhs, start=start, stop=stop)


def _patch_act_tables():
    """Constrain the act-table chooser: Exp/Ln resolve only to the combined
    natural_log_exp table, Silu only to silu_and_others, so the scheduler's
    Exp/Ln interleaving stops thrashing table loads. Table ids stay
    positional; the real tables do contain the claimed functions."""
    import concourse.bacc as _bacc
    import concourse.hw_specs as _hw
    if getattr(_bacc, "_act_tables_patched", False):
        return
    orig = _hw.get_activation_tables

    def patched(module_arch):
        tabs = orig(module_arch)
        keep_exp_ln = "natural_log_exp_and_others"
        out = {}
        for name, funcs in tabs.items():
            f = set(funcs)
            if name != keep_exp_ln:
                f.discard(ACTF.Exp)
                f.discard(ACTF.Ln)
            out[name] = f
        return out

    _bacc.get_activation_tables = patched
    _bacc._act_tables_patched = True


def build_program():
    _patch_act_tables()
    nc = bacc.Bacc("TRN2", target_bir_lowering=False, debug=False,
                   num_devices=N_CORES)

    def din(name, shape, dtype):
        return nc.dram_tensor(name, shape, dtype, kind="ExternalInput")

    # ---- per-core external inputs (host-prepped layouts) ----
    x_cols_d = din("x_cols", [28, NPIX1], BF16)
    c1_lhsT_d = din("c1_lhsT", [28, 128], BF16)
    gn1_wb_d = din("gn1_wb", [128, 2], F32)
    c2_lhsT_d = din("c2_lhsT", [128, 9 * 128], BF16)
    c2_b_d = din("c2_b", [1, 128], BF16)
    gn2_wb_d = din("gn2_wb", [128, 2], F32)
    gb16_d = din("gb16", [128, 128], F32)
    gb32_d = din("gb32", [128, 128], F32)
    ones128_d = din("ones128", [128, 128], F32)
    yred_d = din("yred_lhsT", [128, 16 * 128], BF16)
    ipw_d = [din(f"ipw_{l}", [128, 2 * 512], BF16) for l in range(NL)]
    xzb_d = [din(f"xzb_{l}", [128, 4], F32) for l in range(NL)]
    cwdg_d = [din(f"cwdg_{l}", [128, 8 * 128], BF16) for l in range(NL)]
    cb_d = [din(f"cb_{l}", [128, 2], F32) for l in range(NL)]
    xpw_d = [din(f"xpw_{l}", [128, 4 * 272], BF16) for l in range(NL)]
    dpw_d = [din(f"dpw_{l}", [16, 256], BF16) for l in range(NL)]
    dpb_d = [din(f"dpb_{l}", [128, 2], F32) for l in range(NL)]
    dalh_d = [din(f"dalh_{l}", [128, 32 * 128], BF16) for l in range(NL)]
    acol_d = [din(f"acol_{l}", [128, 32], F32) for l in range(NL)]
    dd_d = [din(f"dd_{l}", [128, 2], F32) for l in range(NL)]
    opw_d = [din(f"opw_{l}", [128, 2 * 256], BF16) for l in range(NL)]

    s_out_d = nc.dram_tensor("s_out", [HID, L], F32, kind="ExternalOutput")

    with tile.TileContext(nc) as tc:
        with tc.tile_pool(name="persist", bufs=1) as pp:
            gb16 = pp.tile([128, 128], F32, tag="gb16")
            nc.sync.dma_start(gb16[:], gb16_d.ap())
            gb32 = pp.tile([128, 128], F32, tag="gb32")
            nc.sync.dma_start(gb32[:], gb32_d.ap())
            ones128 = pp.tile([128, 128], F32, tag="ones128")
            nc.sync.dma_start(ones128[:], ones128_d.ap())
            yred = pp.tile([128, 16 * 128], BF16, tag="yred")
            nc.sync.dma_start(yred[:], yred_d.ap())
            ones_row = pp.tile([1, N5], BF16, tag="ones_row")
            nc.vector.memset(ones_row[:], 1.0)
            eps_col = pp.tile([128, 1], F32, tag="eps_col")
            nc.vector.memset(eps_col[:], EPS)

            # residual stream sT [HID, L] fp32 as 2 partition tiles
            s_t = [pp.tile([128, L], F32, tag=f"s{k}", name=f"s_{k}") for k in range(2)]

            with tc.tile_pool(name="dram", bufs=1, space="DRAM") as dp:
                s_cc_in = dp.tile([128, L], F32, tag="s_cc_in")
                s_cc_out = dp.tile([HID, L], F32, tag="s_cc_out")
                u_cc_in = dp.tile([DH, L], BF16, tag="u_cc_in")
                u_cc_out = dp.tile([DIN, L], BF16, tag="u_cc_out")
                o_cc_in = dp.tile([NTCH * HID, TCH], F32, tag="o_cc_in")
                o_cc_out = dp.tile([NTCH * HID, TCH], F32, tag="o_cc_out")

                build_conv_stem(nc, tc, x_cols_d, c1_lhsT_d, gn1_wb_d,
                                c2_lhsT_d, c2_b_d, gn2_wb_d, gb16, gb32,
                                ones_row, eps_col, s_t, s_cc_in, s_cc_out)

                for l in range(NL):
                    build_layer(nc, tc, l, s_t, ones128, yred, eps_col,
                                ipw_d[l], xzb_d[l], cwdg_d[l], cb_d[l],
                                xpw_d[l],
                                dpw_d[l], dpb_d[l], dalh_d[l], acol_d[l],
                                dd_d[l],
                                opw_d[l], u_cc_in, u_cc_out, o_cc_in,
                                o_cc_out)

            for k in range(2):
                nc.sync.dma_start(s_out_d.ap()[k * 128:(k + 1) * 128, :],
                                  s_t[k][:])

    nc.compile()
    return nc


def group_norm_scale_bias(nc, pool, gsum_ps, wb, inv_n, eps_col):
    """From group-replicated [128,2] psum (sum, sumsq) -> per-partition
    scale/bias columns for the fused silu(gn) activation."""
    m = pool.tile([128, 1], F32, tag="gn_m")
    nc.vector.tensor_scalar(m[:], gsum_ps[:, 0:1], inv_n, None, ALU.mult)
    m2 = pool.tile([128, 1], F32, tag="gn_m2")
    nc.scalar.square(m2[:], m[:])
    var = pool.tile([128, 1], F32, tag="gn_var")
    nc.vector.scalar_tensor_tensor(var[:], gsum_ps[:, 1:2], inv_n, m2[:],
                                   ALU.mult, ALU.subtract)
    lnv = pool.tile([128, 1], F32, tag="gn_lnv")
    nc.scalar.activation(lnv[:], var[:], ACTF.Ln, bias=eps_col[:])
    inv = pool.tile([128, 1], F32, tag="gn_inv")
    nc.scalar.activation(inv[:], lnv[:], ACTF.Exp, scale=-0.5)
    scale = pool.tile([128, 1], F32, tag="gn_scale")
    nc.vector.tensor_tensor(scale[:], inv[:], wb[:, 0:1], ALU.mult)
    nscale = pool.tile([128, 1], F32, tag="gn_nscale")
    nc.vector.tensor_scalar(nscale[:], scale[:], -1.0, None, ALU.mult)
    bias = pool.tile([128, 1], F32, tag="gn_bias")
    nc.vector.scalar_tensor_tensor(bias[:], m[:], nscale[:], wb[:, 1:2],
                                   ALU.mult, ALU.add)
    return scale, bias


def build_conv_stem(nc, tc, x_cols_d, c1_lhsT_d, gn1_wb_d, c2_lhsT_d, c2_b_d,
                    gn2_wb_d, gb16, gb32, ones_row, eps_col, s_t, s_cc_in,
                    s_cc_out):
    from contextlib import ExitStack
    es = ExitStack()
    cp = es.enter_context(tc.tile_pool(name="conv_w", bufs=1))
    psp = es.enter_context(tc.tile_pool(name="conv_ps", bufs=2, space="PSUM"))
    scp = es.enter_context(tc.tile_pool(name="conv_sc", bufs=2))
    c1_lhsT = cp.tile([28, 128], BF16, tag="c1w")
    nc.sync.dma_start(c1_lhsT[:], c1_lhsT_d.ap())
    gn1_wb = cp.tile([128, 2], F32, tag="gn1wb")
    nc.sync.dma_start(gn1_wb[:], gn1_wb_d.ap())

    sums = cp.tile([128, 32], F32, tag="c1_sums")
    sqs = cp.tile([128, 32], F32, tag="c1_sqs")

    es_y1 = ExitStack()
    padp = es_y1.enter_context(tc.tile_pool(name="conv_pad0", bufs=1))
    # padded layout [128, 129*129]: row 0 / col 0 are zero padding
    y1pad = padp.tile([128, 129 * 129], BF16, tag="y1pad")
    y1p = y1pad[:].rearrange("p (h w) -> p h w", h=129)
    nc.vector.memset(y1p[:, 0:1, :], 0.0)
    nc.vector.memset(y1p[:, :, 0:1], 0.0)
    with tc.tile_pool(name="conv_x", bufs=1) as xp_pool:
        x_cols = xp_pool.tile([28, NPIX1], BF16, tag="x_cols")
        nc.sync.dma_start(x_cols[:], x_cols_d.ap())
        for n in range(NPIX1 // N5):
            ps = psp.tile([128, N5], F32, tag="c_ps")
            _mm(nc, ps[:], c1_lhsT[:], x_cols[:, n * N5:(n + 1) * N5])
            r0, c0 = divmod(n * N5, 128)
            dst = y1p[:, 1 + r0:1 + r0 + 4, 1:129]
            nc.scalar.activation(dst, ps[:], ACTF.Copy,
                                 accum_out=sums[:, n:n + 1])
            scr = scp.tile([128, N5], F32, tag="c_scr")
            nc.scalar.activation(scr[:], ps[:], ACTF.Square,
                                 accum_out=sqs[:, n:n + 1])

    stats = cp.tile([128, 2], F32, tag="c_stats")
    nc.vector.tensor_reduce(stats[:, 0:1], sums[:], AX.X, ALU.add)
    nc.vector.tensor_reduce(stats[:, 1:2], sqs[:], AX.X, ALU.add)
    gs_ps = psp.tile([128, 2], F32, tag="c_gs")
    _mm(nc, gs_ps[:], gb16[:], stats[:])
    scale, bias = group_norm_scale_bias(nc, cp, gs_ps, gn1_wb,
                                        1.0 / (16 * NPIX1), eps_col)

    if True:
        gn1_view = y1p[:, 1:129, 1:129]
        nc.scalar.activation(gn1_view, gn1_view, ACTF.Silu,
                             bias=bias[:], scale=scale[:])

        # ---- conv2 (oc half per core), stride 2, 3x3 ----
        c2_lhsT = cp.tile([128, 9 * 128], BF16, tag="c2w")
        nc.sync.dma_start(c2_lhsT[:], c2_lhsT_d.ap())
        c2_b = cp.tile([1, 128], BF16, tag="c2b")
        nc.sync.dma_start(c2_b[:], c2_b_d.ap())
        gn2_wb = cp.tile([128, 2], F32, tag="gn2wb")
        nc.sync.dma_start(gn2_wb[:], gn2_wb_d.ap())

        y2_raw = cp.tile([128, L], F32, tag="y2_raw")
        sums2 = cp.tile([128, 8], F32, tag="c2_sums")
        sqs2 = cp.tile([128, 8], F32, tag="c2_sqs")
        for n in range(L // N5):
            oh0 = n * 8
            ps = psp.tile([128, N5], F32, tag="c_ps")
            first = True
            for kh in range(3):
                for kw in range(3):
                    rhs = y1p[:, kh + 2 * oh0: kh + 2 * oh0 + 15: 2,
                              kw: kw + 127: 2]
                    _mm(nc, ps[:], c2_lhsT[:, (kh * 3 + kw) * 128:
                                           (kh * 3 + kw + 1) * 128],
                        rhs, start=first, stop=False)
                    first = False
            _mm(nc, ps[:], c2_b[:], ones_row[:], start=False, stop=True)
            nc.scalar.activation(y2_raw[:, n * N5:(n + 1) * N5], ps[:],
                                 ACTF.Copy, accum_out=sums2[:, n:n + 1])
            scr = scp.tile([128, N5], F32, tag="c_scr")
            nc.scalar.activation(scr[:], ps[:], ACTF.Square,
                                 accum_out=sqs2[:, n:n + 1])
        es_y1.close()  # y1pad dead

    stats2 = cp.tile([128, 2], F32, tag="c_stats2")
    nc.vector.tensor_reduce(stats2[:, 0:1], sums2[:], AX.X, ALU.add)
    nc.vector.tensor_reduce(stats2[:, 1:2], sqs2[:], AX.X, ALU.add)
    gs2_ps = psp.tile([128, 2], F32, tag="c_gs")
    _mm(nc, gs2_ps[:], gb32[:], stats2[:])
    scale2, bias2 = group_norm_scale_bias(nc, cp, gs2_ps, gn2_wb,
                                          1.0 / (32 * L), eps_col)
    nc.scalar.activation(y2_raw[:], y2_raw[:], ACTF.Silu, bias=bias2[:],
                         scale=scale2[:])
    s_own = y2_raw

    # exchange halves -> full sT on both cores of the pair
    nc.sync.dma_start(s_cc_in[:], s_own[:])
    nc.gpsimd.collective_compute(
        "AllGather", ALU.bypass, replica_groups=GROUPS,
        ins=[s_cc_in.opt()], outs=[s_cc_out.opt()])
    for k in range(2):
        nc.sync.dma_start(s_t[k][:], s_cc_out[k * 128:(k + 1) * 128, :])
    es.close()


def build_layer(nc, tc, l, s_t, ones128, yred, eps_col, ipw_d, xzb_d, cwdg_d,
                cb_d, xpw_d,
                dpw_d, dpb_d, dalh_d, acol_d, dd_d, opw_d, u_cc_in,
                u_cc_out, o_cc_in, o_cc_out):
    with tc.tile_pool(name=f"lw{l}", bufs=1) as wp:
        xzb = wp.tile([128, 4], F32, tag="xzb")
        nc.sync.dma_start(xzb[:], xzb_d.ap())
        cwdg = wp.tile([128, 8 * 128], BF16, tag="cwdg")
        nc.sync.dma_start(cwdg[:], cwdg_d.ap())
        cb = wp.tile([128, 2], F32, tag="cb")
        nc.sync.dma_start(cb[:], cb_d.ap())
        xpw = wp.tile([128, 4 * 272], BF16, tag="xpw")
        nc.sync.dma_start(xpw[:], xpw_d.ap())
        dpw = wp.tile([16, 256], BF16, tag="dpw")
        nc.sync.dma_start(dpw[:], dpw_d.ap())
        dpb = wp.tile([128, 2], F32, tag="dpb")
        nc.sync.dma_start(dpb[:], dpb_d.ap())
        dalh = wp.tile([128, 32 * 128], BF16, tag="dalh")
        nc.sync.dma_start(dalh[:], dalh_d.ap())
        dd = wp.tile([128, 2], F32, tag="dd")
        nc.sync.dma_start(dd[:], dd_d.ap())
        opw = wp.tile([128, 2 * 256], BF16, tag="opw")
        nc.sync.dma_start(opw[:], opw_d.ap())

        with tc.tile_pool(name=f"lact{l}", bufs=1) as ap:
            z_t = [ap.tile([128, L], BF16, tag=f"z{k}", name=f"z_{l}_{k}") for k in range(2)]
            u_own = [ap.tile([128, L], BF16, tag=f"uo{k}", name=f"uo_{l}_{k}") for k in range(2)]
            hstate = ap.tile([128, NG], F32, tag="hstate")
            bt = ap.tile([128, L], BF16, tag="bt")
            ct = ap.tile([128, L], BF16, tag="ct")
            dtt = ap.tile([16, L], BF16, tag="dtt")

            # ---- phase A: LN + in_proj + conv1d + u-gather ----
            phase_a(nc, tc, l, s_t, ones128, eps_col, ipw_d, xzb, cwdg, cb,
                    u_own, z_t, u_cc_in, u_cc_out)

            # ---- phase B: x_proj -> dt | B~ | C~ (whole L) ----
            with tc.tile_pool(name=f"B{l}", bufs=2) as pb, \
                 tc.tile_pool(name=f"Bps{l}", bufs=2, space="PSUM") as pbp:
                for n in range(L // N5):
                    c = slice(n * N5, (n + 1) * N5)
                    us = []
                    for k in range(4):
                        ut = pb.tile([128, N5], BF16, tag=f"us{k}")
                        nc.sync.dma_start(
                            ut[:], u_cc_out[k * 128:(k + 1) * 128, c])
                        us.append(ut)
                    for mi, (m0, m1, dst) in enumerate(
                            [(0, 16, dtt), (16, 144, bt), (144, 272, ct)]):
                        ps = pbp.tile([m1 - m0, N5], F32, tag=f"ps{mi}")
                        for k in range(4):
                            _mm(nc, ps[:],
                                xpw[:, k * 272 + m0:k * 272 + m1], us[k][:],
                                start=(k == 0), stop=(k == 3))
                        nc.scalar.activation(dst[:, c], ps[:], ACTF.Copy)

            # ---- phase C: delta + scan stack ----
            phase_c(nc, tc, l, dtt, dpw, dpb, bt, ct, hstate, dalh, yred,
                    dd, opw, u_own, z_t, o_cc_in, o_cc_out, s_t)



def phase_a(nc, tc, l, s_t, ones128, eps_col, ipw_d, xzb, cwdg, cb, u_own,
            z_t, u_cc_in, u_cc_out):
    with tc.tile_pool(name=f"A{l}", bufs=2) as pa, \
         tc.tile_pool(name=f"Ab{l}", bufs=1) as pa1, \
         tc.tile_pool(name=f"Aps{l}", bufs=2, space="PSUM") as pap:
        ipw = pa1.tile([128, 2 * 512], BF16, tag="ipw")
        nc.sync.dma_start(ipw[:], ipw_d.ap())
        xc_lin = [pa1.tile([128, 3 + L], BF16, tag=f"xcl{k}",
                               name=f"xcl_{l}_{k}") for k in range(2)]
        for k in range(2):
            nc.vector.memset(xc_lin[k][:, 0:3], 0.0)
        for n in range(L // N5):
            c = slice(n * N5, (n + 1) * N5)
            ps_m = pap.tile([128, N5], F32, tag="ln_m")
            _mm(nc, ps_m[:], ones128[:], s_t[0][:, c], start=True,
                stop=False)
            _mm(nc, ps_m[:], ones128[:], s_t[1][:, c], start=False,
                stop=True)
            ps_q = pap.tile([128, N5], F32, tag="ln_q")
            for k in range(2):
                sq = pa.tile([128, N5], F32, tag="ln_sq")
                nc.vector.tensor_tensor(sq[:], s_t[k][:, c], s_t[k][:, c],
                                        ALU.mult)
                _mm(nc, ps_q[:], ones128[:], sq[:], start=(k == 0),
                    stop=(k == 1))
            m2s = pa.tile([128, N5], F32, tag="ln_m2s")
            nc.scalar.activation(m2s[:], ps_m[:], ACTF.Square,
                                 scale=1.0 / HID)
            var = pa.tile([128, N5], F32, tag="ln_var")
            nc.vector.scalar_tensor_tensor(var[:], ps_q[:], 1.0 / HID,
                                           m2s[:], ALU.mult, ALU.subtract)
            lnv = pa.tile([128, N5], F32, tag="ln_lnv")
            nc.scalar.activation(lnv[:], var[:], ACTF.Ln, bias=eps_col[:])
            inv = pa.tile([128, N5], F32, tag="ln_inv")
            nc.scalar.activation(inv[:], lnv[:], ACTF.Exp, scale=-0.5)
            xn = []
            for k in range(2):
                t1 = pa.tile([128, N5], F32, tag="ln_t1")
                nc.vector.scalar_tensor_tensor(
                    t1[:], ps_m[:], -1.0 / HID, s_t[k][:, c], ALU.mult,
                    ALU.add)
                xnk = pa.tile([128, N5], BF16, tag="ln_xn")
                nc.vector.tensor_tensor(xnk[:], t1[:], inv[:], ALU.mult)
                xn.append(xnk)
            # in_proj: m-tiles 0,1 = xc halves; 2,3 = z halves
            for m in range(4):
                ps = pap.tile([128, N5], F32, tag="xz_ps")
                for k in range(2):
                    _mm(nc, ps[:],
                        ipw[:, k * 512 + m * 128:k * 512 + (m + 1) * 128],
                        xn[k][:], start=(k == 0), stop=(k == 1))
                if m < 2:
                    nc.scalar.activation(
                        xc_lin[m][:, 3 + n * N5: 3 + (n + 1) * N5], ps[:],
                        ACTF.Identity, bias=xzb[:, m:m + 1])
                else:
                    nc.scalar.activation(z_t[m - 2][:, c], ps[:],
                                         ACTF.Identity,
                                         bias=xzb[:, m:m + 1])
        # conv1d (depthwise, 4 causal taps) as PE diagonal matmuls,
        # silu fused into the psum evacuation
        for k in range(2):
            for n in range(L // N5):
                ps_u = pap.tile([128, N5], F32, tag="cv_ps")
                for j in range(4):
                    _mm(nc, ps_u[:],
                        cwdg[:, (k * 4 + j) * 128:(k * 4 + j + 1) * 128],
                        xc_lin[k][:, j + n * N5: j + (n + 1) * N5],
                        start=(j == 0), stop=(j == 3))
                nc.scalar.activation(u_own[k][:, n * N5:(n + 1) * N5],
                                     ps_u[:], ACTF.Identity,
                                     bias=cb[:, k:k + 1])
        for k in range(2):
            nc.scalar.activation(u_own[k][:], u_own[k][:], ACTF.Silu)
            nc.scalar.activation(z_t[k][:], z_t[k][:], ACTF.Silu)
            nc.sync.dma_start(u_cc_in[k * 128:(k + 1) * 128, :],
                              u_own[k][:])
    nc.gpsimd.collective_compute(
        "AllGather", ALU.bypass, replica_groups=GROUPS,
        ins=[u_cc_in.opt()], outs=[u_cc_out.opt()])


def phase_c(nc, tc, l, dtt, dpw, dpb, bt, ct, hstate, dalh, yred, dd, opw,
            u_own, z_t, o_cc_in, o_cc_out, s_t):
    with tc.tile_pool(name=f"C{l}", bufs=3) as pc, \
         tc.tile_pool(name=f"Ch{l}", bufs=9) as phc, \
         tc.tile_pool(name=f"Cu{l}", bufs=6) as pdur, \
         tc.tile_pool(name=f"Cd{l}", bufs=2) as pdl, \
         tc.tile_pool(name=f"Cr{l}", bufs=2) as pr, \
         tc.tile_pool(name=f"Cps{l}", bufs=3, space="PSUM") as pcp, \
         tc.tile_pool(name=f"Cp1{l}", bufs=1, space="PSUM") as pcp1, \
         tc.tile_pool(name=f"Cyps{l}", bufs=1, space="PSUM") as pyp:

        def delta_du(t):
            ts_l = slice(t * TCH, (t + 1) * TCH)
            dl, duc = [], []
            for k in range(2):
                dlk = pdl.tile([128, TCH], BF16, tag=f"dl{k}",
                               name=f"dl_{l}_{t}_{k}")
                for j in range(TCH // N5):
                    cj = slice(t * TCH + j * N5, t * TCH + (j + 1) * N5)
                    ps_d = pcp1.tile([128, N5], F32, tag="ps_so")
                    _mm(nc, ps_d[:], dpw[:, k * 128:(k + 1) * 128],
                        dtt[:, cj])
                    es_t = pdl.tile([128, N5], F32, tag="sp_e")
                    nc.scalar.activation(es_t[:], ps_d[:], ACTF.Exp,
                                         bias=dpb[:, k:k + 1])
                    nc.scalar.activation(dlk[:, j * N5:(j + 1) * N5],
                                         es_t[:], ACTF.Ln, bias=1.0)
                duk = pdl.tile([128, TCH], BF16, tag=f"du{k}",
                               name=f"du_{l}_{t}_{k}")
                nc.vector.tensor_tensor(duk[:], dlk[:], u_own[k][:, ts_l],
                                        ALU.mult)
                dl.append(dlk)
                duc.append(duk)
            return dl, duc

        nxt = delta_du(0)
        deferred = []
        for t in range(NTCH):
            ts = slice(t * TCH, (t + 1) * TCH)
            dl, duc = nxt
            if t + 1 < NTCH:
                nxt = delta_du(t + 1)
            ps_y = [pyp.tile([128, TCH], F32, tag=f"ps_y{pt}",
                             name=f"ps_y_{l}_{t}_{pt}") for pt in range(2)]
            hc_batch = []
            for g in range(NG):
                kt = g // 16
                r0 = (8 * g) % 128
                # dA = exp(A * delta) via one-hot*A matmul + exp
                da = pc.tile([128, TCH], BF16, tag="da")
                for j in range(TCH // N5):
                    ps_da = pcp.tile([128, N5], F32, tag="ps_da")
                    _mm(nc, ps_da[:],
                        dalh[:, g * 128:(g + 1) * 128],
                        dl[kt][:, j * N5:(j + 1) * N5])
                    nc.scalar.activation(da[:, j * N5:(j + 1) * N5],
                                         ps_da[:], ACTF.Exp)
                # du_rep via broadcast DMA, dBu = du_rep * B~
                dur = pdur.tile([128, TCH], BF16, tag="dur")
                src = duc[kt][r0:r0 + 8, :].unsqueeze(1)
                nc.gpsimd.dma_start(dur[:], src.broadcast_to([8, 16, TCH]))
                dbu = pc.tile([128, TCH], BF16, tag="dbu")
                nc.vector.tensor_tensor(dbu[:], dur[:], bt[:, ts], ALU.mult)
                # selective scan
                h = pc.tile([128, TCH], BF16, tag="h")
                init = 0.0 if t == 0 else hstate[:, g:g + 1]
                nc.vector.tensor_tensor_scan(h[:], da[:], dbu[:], init,
                                             ALU.mult, ALU.add)
                nc.vector.tensor_copy(hstate[:, g:g + 1],
                                      h[:, TCH - 1:TCH])
                # y partial: sum_n h*C; yred matmuls emitted in batches of
                # 8 lane-tiles so the PE stream is never head-of-line
                # blocked waiting for a single hc
                hc = phc.tile([128, TCH], BF16, tag="hc")
                nc.vector.tensor_tensor(hc[:], h[:], ct[:, ts], ALU.mult)
                hc_batch.append((g, kt, hc))
                if len(hc_batch) == 8:
                    for gg, kkt, hhc in hc_batch:
                        for j in range(TCH // N5):
                            _mm(nc, ps_y[kkt][:, j * N5:(j + 1) * N5],
                                yred[:, (gg % 16) * 128:
                                     (gg % 16 + 1) * 128],
                                hhc[:, j * N5:(j + 1) * N5],
                                start=(gg % 16 == 0), stop=(gg % 16 == 15))
                    hc_batch = []
            # y = ys + u*D ; r = y * silu(z) ; out_proj partial
            rt = []
            for k in range(2):
                y = pr.tile([128, TCH], BF16, tag=f"y{k}")
                nc.vector.scalar_tensor_tensor(
                    y[:], u_own[k][:, ts], dd[:, k:k + 1], ps_y[k][:],
                    ALU.mult, ALU.add)
                r = pr.tile([128, TCH], BF16, tag=f"r{k}")
                nc.vector.tensor_tensor(r[:], y[:], z_t[k][:, ts], ALU.mult)
                rt.append(r)
            for m in range(2):
                for j in range(TCH // N5):
                    ps_o = pcp1.tile([128, N5], F32, tag="ps_so")
                    for k in range(2):
                        _mm(nc, ps_o[:],
                            opw[:, k * 256 + m * 128:
                                k * 256 + (m + 1) * 128],
                            rt[k][:, j * N5:(j + 1) * N5],
                            start=(k == 0), stop=(k == 1))
                    o_sb = pr.tile([128, N5], F32, tag="o_sb")
                    nc.scalar.activation(o_sb[:], ps_o[:], ACTF.Copy)
                    nc.sync.dma_start(
                        o_cc_in[t * HID + m * 128:t * HID + (m + 1) * 128,
                                j * N5:(j + 1) * N5],
                        o_sb[:])
            # launch the pair all-reduce now; apply residual at layer end
            nc.gpsimd.collective_compute(
                "AllReduce", ALU.add, replica_groups=GROUPS,
                ins=[o_cc_in[t * HID:(t + 1) * HID, :].opt()],
                outs=[o_cc_out[t * HID:(t + 1) * HID, :].opt()])
            deferred.append(t)
        for t in deferred:
            ts = slice(t * TCH, (t + 1) * TCH)
            for k in range(2):
                o = pr.tile([128, TCH], F32, tag="o_in")
                nc.gpsimd.dma_start(
                    o[:], o_cc_out[t * HID + k * 128:
                                   t * HID + (k + 1) * 128, :])
                nc.vector.tensor_tensor(s_t[k][:, ts], s_t[k][:, ts],
                                        o[:], ALU.add)


# ======================= host side =======================

def _prep_inputs(inputs):
    """Host-side weight layout prep. Returns per-core input maps."""
    f32 = np.float32
    import ml_dtypes
    bf16 = ml_dtypes.bfloat16

    def to_bf(a):
        return np.asarray(a, dtype=f32).astype(bf16)

    x = np.asarray(inputs["x"], f32)
    conv1_w = np.asarray(inputs["conv1_w"], f32)
    conv1_b = np.asarray(inputs["conv1_b"], f32)
    gn1_w = np.asarray(inputs["gn1_w"], f32)
    gn1_b = np.asarray(inputs["gn1_b"], f32)
    conv2_w = np.asarray(inputs["conv2_w"], f32)
    conv2_b = np.asarray(inputs["conv2_b"], f32)
    gn2_w = np.asarray(inputs["gn2_w"], f32)
    gn2_b = np.asarray(inputs["gn2_b"], f32)
    ln_w = np.asarray(inputs["ln_w"], f32)
    ln_b = np.asarray(inputs["ln_b"], f32)
    in_proj_w = np.asarray(inputs["in_proj_w"], f32)
    conv1d_w = np.asarray(inputs["conv1d_w"], f32)
    conv1d_b = np.asarray(inputs["conv1d_b"], f32)
    x_proj_w = np.asarray(inputs["x_proj_w"], f32)
    dt_proj_w = np.asarray(inputs["dt_proj_w"], f32)
    dt_proj_b = np.asarray(inputs["dt_proj_b"], f32)
    A_log = np.asarray(inputs["A_log"], f32)
    Dp = np.asarray(inputs["D"], f32)
    out_proj_w = np.asarray(inputs["out_proj_w"], f32)

    # --- conv1 im2col per sample: rows (c,kh,kw) + ones row ---
    xp = np.zeros((B, 3, H + 2, W + 2), f32)
    xp[:, :, 1:H + 1, 1:W + 1] = x
    cols = []
    for c in range(3):
        for kh in range(3):
            for kw in range(3):
                v = xp[:, c, kh:kh + 2 * 128:2, kw:kw + 2 * 128:2]
                cols.append(v.reshape(B, -1))
    x_cols = np.stack(cols, axis=1)  # [B, 27, 16384]
    x_cols = np.concatenate(
        [x_cols, np.ones((B, 1, NPIX1), f32)], axis=1)

    c1_lhsT = np.concatenate(
        [conv1_w.reshape(128, 27).T, conv1_b[None, :]], axis=0)  # [28,128]
    gn1_wb = np.stack([gn1_w, gn1_b], axis=1)

    def gblock(gsz):
        m = np.zeros((128, 128), f32)
        for i in range(128):
            g0 = (i // gsz) * gsz
            m[g0:g0 + gsz, i] = 1.0
        return m

    gb16, gb32 = gblock(16), gblock(32)
    ones128 = np.ones((128, 128), f32)

    yred = np.zeros((16, 128, 128), f32)
    for j in range(16):
        for p in range(128):
            yred[j, p, 8 * j + p // 16] = 1.0
    yred_l = to_bf(yred.transpose(1, 0, 2).reshape(128, 16 * 128))

    shared = dict(gb16=gb16, gb32=gb32, ones128=ones128, yred_lhsT=yred_l)

    in_maps = []
    for core in range(N_CORES):
        b, hh = core // 2, core % 2
        dsl = slice(hh * DH, (hh + 1) * DH)
        m = dict(shared)
        m["x_cols"] = to_bf(x_cols[b])
        m["c1_lhsT"] = to_bf(c1_lhsT)
        m["gn1_wb"] = gn1_wb
        ocs = slice(hh * 128, (hh + 1) * 128)
        c2 = np.zeros((128, 9 * 128), f32)
        for kh in range(3):
            for kw in range(3):
                tap = kh * 3 + kw
                c2[:, tap * 128:(tap + 1) * 128] = conv2_w[ocs, :, kh, kw].T
        m["c2_lhsT"] = to_bf(c2)
        m["c2_b"] = to_bf(conv2_b[None, ocs])
        m["gn2_wb"] = np.stack([gn2_w[ocs], gn2_b[ocs]], axis=1)

        for l in range(NL):
            ipw = in_proj_w[l] * ln_w[l][None, :]
            xzb = in_proj_w[l] @ ln_b[l]
            rows = np.concatenate(
                [np.arange(hh * DH, (hh + 1) * DH),
                 np.arange(DIN + hh * DH, DIN + (hh + 1) * DH)])
            ipw_h = ipw[rows]
            ipw_l = np.zeros((128, 2 * 512), f32)
            for k in range(2):
                ipw_l[:, k * 512:(k + 1) * 512] = \
                    ipw_h[:, k * 128:(k + 1) * 128].T
            m[f"ipw_{l}"] = to_bf(ipw_l)
            xzb_h = xzb[rows]
            m[f"xzb_{l}"] = np.stack(
                [xzb_h[0:128], xzb_h[128:256], xzb_h[256:384],
                 xzb_h[384:512]], axis=1)
            cw = conv1d_w[l, dsl, 0, :]
            cbv = conv1d_b[l, dsl]
            cwdg = np.zeros((128, 8 * 128), f32)
            for k in range(2):
                for j in range(4):
                    cwdg[:, (k * 4 + j) * 128:(k * 4 + j + 1) * 128] = \
                        np.diag(cw[k * 128:(k + 1) * 128, j])
            m[f"cwdg_{l}"] = to_bf(cwdg)
            m[f"cb_{l}"] = np.stack([cbv[0:128], cbv[128:256]], axis=1)
            xa = np.zeros((DIN, 272), f32)
            xa[:, 0:16] = x_proj_w[l, 0:DTR, :].T
            nidx = np.arange(128) % 16
            xa[:, 16:144] = x_proj_w[l, DTR + nidx, :].T
            xa[:, 144:272] = x_proj_w[l, DTR + DST + nidx, :].T
            xpw_l = np.zeros((128, 4 * 272), f32)
            for k in range(4):
                xpw_l[:, k * 272:(k + 1) * 272] = xa[k * 128:(k + 1) * 128]
            m[f"xpw_{l}"] = to_bf(xpw_l)
            m[f"dpw_{l}"] = to_bf(dt_proj_w[l, dsl, :].T)
            dpb_h = dt_proj_b[l, dsl]
            m[f"dpb_{l}"] = np.stack([dpb_h[0:128], dpb_h[128:256]], axis=1)
            A = -np.exp(A_log[l, dsl, :])
            dal = np.zeros((32, 128, 128), f32)
            for g in range(32):
                r0 = (8 * g) % 128
                for p in range(128):
                    dal[g, r0 + p // 16, p] = A[8 * g + p // 16, p % 16]
            m[f"dalh_{l}"] = to_bf(
                dal.transpose(1, 0, 2).reshape(128, 32 * 128))
            ac = np.zeros((128, 32), f32)
            for g in range(32):
                for p in range(128):
                    ac[p, g] = A[8 * g + p // 16, p % 16]
            m[f"acol_{l}"] = ac
            Dh = Dp[l, dsl]
            m[f"dd_{l}"] = np.stack([Dh[0:128], Dh[128:256]], axis=1)
            opw_h = out_proj_w[l][:, dsl]
            opw_l = np.zeros((128, 2 * 256), f32)
            for k in range(2):
                opw_l[:, k * 256:(k + 1) * 256] = \
                    opw_h[:, k * 128:(k + 1) * 128].T
            m[f"opw_{l}"] = to_bf(opw_l)
        in_maps.append({k: np.ascontiguousarray(v) for k, v in m.items()})
    return in_maps


def kernel(**inputs):
    if "nc" not in _cache:
        _cache["nc"] = build_program()
    nc = _cache["nc"]
    in_maps = _prep_inputs(inputs)
    res = run_bass_kernel_spmd(nc, in_maps, list(range(N_CORES)))
    s = np.stack([np.asarray(res.results[2 * b]["s_out"], np.float32).T
                  for b in range(B)])
    return (s, 64, 64)
